# revision 31
# baseline (speedup 1.0000x reference)
"""LSTM encoder kernel for Trainium2 (8 NeuronCores, data-parallel over batch).

Layout trick: SBUF/PSUM partition p = 32*q + b (q = H-quarter, b = batch row),
so the 32-batch-per-core problem fills all 128 partitions. The recurrent
matmul h @ W_hh.T runs as 4 CONCURRENT col-strip matmuls (PE tile_position
col-tiling, one 32-wide strip per quarter, each streaming its own W slice
over a separate XBUS) — ~4x the PE throughput of a single M=32 matmul chain.
The input projection is injected into the same PSUM accumulation via a
one-hot matmul against a 120-row combined-embedding table XC (30 letter
tokens x 4 state tokens), so no per-step gather DMA and no DVE add.

Per step:   psum[32q+b, n] = OH_t.T @ XC_q  +  sum_k hT[k].T @ W[k]_q
            (4 strips x 2 banks; per-quarter gate col layout [g|i|f|o])
            ACT: tanh(g), sigm(i,f,o), tanh(c');  DVE: c' = f*c + i*g, h = o*th
The h feedback transposes on the PE itself (tensor.transpose vs identity,
~0.3us while the PE is idle waiting for hT anyway; a transpose-DMA costs
~4-5us of latency here) + a DVE copy PSUM->SBUF. One [128,128] transpose
per h-half lands all four strips' k-blocks at once (k = 2q + half); the
half-1 transpose is emitted mid-way through the NEXT step's MM stream so
the in-order PE queue doesn't stall on it, and next-step MMs consume hT
halves in production order (k evens, then odds). The next step's one-hot
xp matmuls are hoisted before the transposes in the PE queue to fill the
tail. Measured ~6.7us/step on TRN2 (PE span ~3.7us + exposed ACT/DVE/PE
feedback chain ~3.0us).
"""

from contextlib import ExitStack

import ml_dtypes
import numpy as np

import concourse.bacc as bacc
import concourse.mybir as mybir
import concourse.tile as tile
from concourse.bass_utils import run_bass_kernel_spmd

F32 = mybir.dt.float32
BF16 = mybir.dt.bfloat16

B, S, E, H = 256, 256, 256, 1024
NCORES = 8
BL = B // NCORES           # 32 batch rows per core
NK = H // 128              # 8 contraction tiles
NQ = 4                     # H quarters == PE col-strips
QH = H // NQ               # 256 h-units per quarter
GATE_OF_BLK = [2, 0, 1, 3]  # per-quarter col blocks [g|i|f|o] -> pytorch i,f,g,o rows
KORDER = [0, 2, 4, 6, 1, 3, 5, 7]  # consume hT blocks in production order (halves)

_cache = {}


def _build(steps: int, repeat: int = 1, bench: bool = False,
           diag_no_feedback: bool = False, diag_no_stores: bool = False,
           diag_no_elementwise: bool = False):
    """Emit the kernel. bench=True wraps the step loop in a hardware For_i
    repeat loop and stores outputs to a small circular buffer (same per-step
    device work, tiny host I/O) for differential wall-clock timing."""
    nc = bacc.Bacc("TRN2", target_bir_lowering=False, debug=False,
                   enable_asserts=not bench)

    w_dram = nc.dram_tensor("W", [H, 4 * H], BF16, kind="ExternalInput")
    xc_dram = nc.dram_tensor("XC", [120, 4 * H], BF16, kind="ExternalInput")
    oh_dram = nc.dram_tensor("OH", [120, BL * steps], BF16, kind="ExternalInput")
    eye_dram = nc.dram_tensor("EYE", [128, 128], BF16, kind="ExternalInput")
    out_steps = 8 if bench else steps
    # outputs in device layout [p=32q+b, t, u]; host reassembles to [BL, t, H]
    hid_dram = nc.dram_tensor("hid", [NQ * BL, out_steps, QH], F32,
                              kind="ExternalOutput")
    cell_dram = nc.dram_tensor("cell", [NQ * BL, out_steps, QH], F32,
                               kind="ExternalOutput")

    Tanh = mybir.ActivationFunctionType.Tanh
    Sigmoid = mybir.ActivationFunctionType.Sigmoid

    hid_view = hid_dram.ap()
    cell_view = cell_dram.ap()

    with tile.TileContext(nc) as tc, ExitStack() as ctx:
        resident = ctx.enter_context(tc.tile_pool(name="resident", bufs=1))
        psum_pool = ctx.enter_context(tc.tile_pool(name="psum", bufs=1, space="PSUM"))

        w_sb = resident.tile([128, NK, 4 * H], BF16)
        w_view = w_dram.ap().rearrange("(k p) n -> k p n", p=128)
        for k in range(NK):
            eng = nc.sync if k % 2 == 0 else nc.scalar
            eng.dma_start(w_sb[:, k], w_view[k])
        xc_sb = resident.tile([120, 4 * H], BF16)
        nc.sync.dma_start(xc_sb[:], xc_dram[:])
        oh_sb = resident.tile([120, BL * steps], BF16)
        nc.scalar.dma_start(oh_sb[:], oh_dram[:])
        eye_sb = resident.tile([128, 128], BF16)
        nc.sync.dma_start(eye_sb[:], eye_dram[:])

        # PSUM: 2 banks (g|i / f|o) x 2 step parities + 2 transpose staging
        ps_st = [[psum_pool.tile([128, 512], F32, name=f"ps{p}{b}") for b in range(2)]
                 for p in range(2)]
        tr_ps = [psum_pool.tile([128, 128], BF16, name=f"tr{h}") for h in range(2)]
        # rotating state (explicit rotation; all periods divide 256)
        c_st = [resident.tile([128, QH], F32, name=f"c{i}") for i in range(4)]
        # hT free layout [half, q, b]: one [128,128] transpose-DMA per half
        # lands all four strips' k-blocks (k = 2q + half) at once
        hT_st = [resident.tile([128, 2, NQ, BL], BF16, name=f"hT{i}") for i in range(2)]
        g_st = [resident.tile([128, QH], F32, name=f"g{i}") for i in range(2)]
        i_st = [resident.tile([128, QH], F32, name=f"i{i}") for i in range(2)]
        f_st = [resident.tile([128, QH], F32, name=f"f{i}") for i in range(2)]
        o_st = [resident.tile([128, QH], F32, name=f"o{i}") for i in range(2)]
        t1_st = [resident.tile([128, QH], F32, name=f"t1{i}") for i in range(2)]
        th_st = [resident.tile([128, QH], F32, name=f"th{i}") for i in range(2)]
        hbf_st = [resident.tile([128, QH], BF16, name=f"hbf{i}") for i in range(4)]

        if diag_no_elementwise:
            diag_no_feedback = True
        if diag_no_feedback:
            # hT never written by the loop; seed once so tiles aren't
            # read-before-write (timing diagnostic only, numerics invalid)
            for st in hT_st:
                for half in range(2):
                    nc.vector.tensor_copy(st[:, half], w_sb[:, 0, 0:128])

        # deferred PE-transpose of half 1: emitted mid-way through the NEXT
        # step's MM stream (so the in-order PE queue doesn't stall on it)
        pending_tr1 = []

        def emit_transpose(t, half):
            hbf = hbf_st[t % 4]
            hT_nxt = hT_st[(t + 1) % 2]
            hs = 128 * half
            nc.tensor.transpose(tr_ps[half][:], hbf[:, hs:hs + 128], eye_sb[:])
            nc.vector.tensor_copy(hT_nxt[:, half], tr_ps[half][:])

        def emit_xp(t):
            # one-hot(idx_t) x XC -> PSUM (opens the accumulation group);
            # hoisted one step early so it fills the PE during step t-1's tail
            par = t % 2
            pa, pb = ps_st[par]
            oh_t = oh_sb[:, BL * t:BL * (t + 1)]
            for beta, pt in ((0, pa), (1, pb)):
                for q in range(NQ):
                    nc.tensor.matmul(
                        pt[32 * q:32 * q + 32, :], oh_t,
                        xc_sb[:, 1024 * q + 512 * beta:1024 * q + 512 * beta + 512],
                        start=True, stop=(t == 0), tile_position=(0, 32 * q))

        def step_body(t):
            par = t % 2
            pa, pb = ps_st[par]
            hT_cur = hT_st[t % 2]

            def mm(pt, beta, k, q):
                nc.tensor.matmul(
                    pt[32 * q:32 * q + 32, :], hT_cur[:, k % 2, k // 2],
                    w_sb[:, k, 1024 * q + 512 * beta:1024 * q + 512 * beta + 512],
                    start=False, stop=(k == KORDER[-1]),
                    tile_position=(0, 32 * q))

            if t == 0:
                emit_xp(0)
            else:
                for beta, pt in ((0, pa), (1, pb)):
                    for k in (0, 2, 4, 6):          # evens: need only hT half 0
                        for q in range(NQ):
                            mm(pt, beta, k, q)
                while pending_tr1:
                    pending_tr1.pop()()              # prev step's half-1 transpose
                for beta, pt in ((0, pa), (1, pb)):
                    for k in (1, 3, 5, 7):
                        for q in range(NQ):
                            mm(pt, beta, k, q)
            if t + 1 < steps:
                emit_xp(t + 1)
            if diag_no_elementwise:
                return

            # elementwise, full 128-partition width
            g_t, i_t = g_st[par], i_st[par]
            f_t, o_t = f_st[par], o_st[par]
            t1 = t1_st[par]
            c_new, c_old = c_st[t % 4], c_st[(t + 3) % 4]
            hbf = hbf_st[t % 4]
            th = th_st[par]

            feedback = (t < steps - 1 or bench) and not diag_no_feedback

            def c_half(hs):
                nc.vector.tensor_mul(c_new[:, hs:hs + 128], f_t[:, hs:hs + 128],
                                     c_old[:, hs:hs + 128])
                nc.vector.tensor_add(c_new[:, hs:hs + 128], c_new[:, hs:hs + 128],
                                     t1[:, hs:hs + 128])

            def h_half(half):
                hs = 128 * half
                nc.scalar.activation(th[:, hs:hs + 128], c_new[:, hs:hs + 128], Tanh)
                nc.vector.tensor_mul(hbf[:, hs:hs + 128], o_t[:, hs:hs + 128],
                                     th[:, hs:hs + 128])
                if feedback:
                    if half == 0:
                        emit_transpose(t, 0)
                    else:
                        pending_tr1.append(lambda tt=t: emit_transpose(tt, 1))

            nc.scalar.activation(g_t[:], pa[:, 0:QH], Tanh)
            nc.scalar.activation(i_t[:], pa[:, QH:2 * QH], Sigmoid)
            if t == 0:
                nc.vector.tensor_mul(c_new[:], i_t[:], g_t[:])
                nc.scalar.activation(o_t[:], pb[:, QH:2 * QH], Sigmoid)
                for half in range(2):
                    h_half(half)
            else:
                nc.vector.tensor_mul(t1[:], i_t[:], g_t[:])
                nc.scalar.activation(f_t[:], pb[:, 0:QH], Sigmoid)
                nc.scalar.activation(o_t[:], pb[:, QH:2 * QH], Sigmoid)
                # critical half-0 chain first; half-1 c ops run after h0 is
                # off to the PE (they'd otherwise delay h0 in the DVE FIFO)
                c_half(0)
                h_half(0)
                c_half(128)
                h_half(1)

            if not diag_no_stores:
                ot = t % out_steps
                nc.gpsimd.dma_start(hid_view[:, ot], hbf[:])  # bf16 -> f32 cast DMA
                nc.scalar.dma_start(cell_view[:, ot], c_new[:])

        if bench and repeat > 1:
            with tc.For_i(0, repeat):
                for t in range(steps):
                    step_body(t)
                while pending_tr1:   # close the feedback before loop back-edge
                    pending_tr1.pop()()
        else:
            for t in range(steps):
                step_body(t)
            pending_tr1.clear()

    nc.compile()
    return nc


def _host_prep(letter_seq, state_seq, letter_emb, state_emb, W_ih, W_hh, b_ih, b_hh,
               steps: int):
    letter_seq = np.asarray(letter_seq)
    state_seq = np.asarray(state_seq)
    letter_emb = np.asarray(letter_emb, dtype=np.float32)
    state_emb = np.asarray(state_emb, dtype=np.float32)
    W_ih = np.asarray(W_ih, dtype=np.float32)
    W_hh = np.asarray(W_hh, dtype=np.float32)
    b_ih = np.asarray(b_ih, dtype=np.float32)
    b_hh = np.asarray(b_hh, dtype=np.float32)

    # new col n = q*1024 + blk*256 + u  ->  orig 4H row (gate-major, i,f,g,o)
    n = np.arange(4 * H)
    q_idx, blk, u = n // 1024, (n % 1024) // QH, n % QH
    colmap = np.array(GATE_OF_BLK)[blk] * H + q_idx * QH + u  # [4H]

    Wp = np.ascontiguousarray(W_hh[colmap, :].T).astype(ml_dtypes.bfloat16)  # [H, 4H]

    XL = letter_emb @ W_ih[:, :E].T                            # [30, 4H]
    XS = state_emb @ W_ih[:, E:].T                             # [4, 4H]
    bias = b_ih + b_hh
    XC = (XL[:, None, :] + XS[None, :, :] + bias).reshape(120, 4 * H)
    XC = np.ascontiguousarray(XC[:, colmap]).astype(ml_dtypes.bfloat16)  # [120, 4H]

    idx = (letter_seq.astype(np.int64) * 4 + state_seq.astype(np.int64))  # [B, S]
    eye = np.eye(128, dtype=ml_dtypes.bfloat16)
    in_maps = []
    for c in range(NCORES):
        idx_c = idx[BL * c:BL * (c + 1), :steps]               # [BL, steps]
        oh = np.zeros((120, BL * steps), dtype=ml_dtypes.bfloat16)
        cols = np.arange(BL * steps)
        oh[idx_c.T.reshape(-1), cols] = 1.0                    # col = t*BL + b
        in_maps.append({"W": Wp, "XC": XC, "OH": oh, "EYE": eye})
    return in_maps


def kernel(letter_seq, state_seq, letter_emb, state_emb, W_ih, W_hh, b_ih, b_hh,
           steps: int = S):
    if steps not in _cache:
        _cache[steps] = _build(steps)
    nc = _cache[steps]

    in_maps = _host_prep(letter_seq, state_seq, letter_emb, state_emb,
                         W_ih, W_hh, b_ih, b_hh, steps)
    res = run_bass_kernel_spmd(nc, in_maps, core_ids=list(range(NCORES)))

    def unshuffle(a):
        # [4q*32b, steps, 256u] -> [32b, steps, 1024h]
        return np.ascontiguousarray(
            a.reshape(NQ, BL, steps, QH).transpose(1, 2, 0, 3).reshape(BL, steps, H)
        )

    hidden = np.concatenate(
        [unshuffle(res.results[c]["hid"]) for c in range(NCORES)], axis=0)
    cell = np.concatenate(
        [unshuffle(res.results[c]["cell"]) for c in range(NCORES)], axis=0)
    return hidden, cell
